# revision 49
# baseline (speedup 1.0000x reference)
"""Trainium2 Bass kernel for per-head attention (TransformerLens-style).

Reference computation (per batch b, head h, with x = resid[b, :, h, :]):
    q = x @ W_Q[h] + b_Q[h];  k = x @ W_K[h] + b_K[h];  v = x @ W_V[h] + b_V[h]
    scores = q @ k.T / sqrt(DH), causal-masked, softmax over keys
    z = P @ v;  out[b, :, h, :] = z @ W_O[h] + b_O / H

Shapes: B=4, S=1024, H=12, DM=768, DH=64.  B*H = 48 independent attention
problems; 8 NeuronCores get 6 each (pure data parallel, no collectives).

Device-side formulation:
  - pairs are grouped into COUPLES sharing a head: couple = (h, batch-half),
    so weights load once per couple and the qk-projection shares one
    Ldweights across both pairs' moving blocks.
  - host passes x^T (DM-major) in bf16; weights bf16, W_Q pre-scaled by
    1/sqrt(DH).  qk^T projection -> psum [128, S] (rows 0:64 q^T, 64:128
    k^T); a partition-swapped copy (swap_sb) lets consecutive score strips
    run ROW-PACKED (PE row groups 0/64 concurrently).
  - scores are computed TRANSPOSED (s^T[sk, sq]) with 128-aligned block
    starts (no wasted pre-diagonal columns); softmax exp runs on ScalarE;
    the diagonal-block causal mask is a bf16 0/1 multiply split between
    VectorE and GpSimd.  Row sums over sk come free via a ones column
    augmenting v (z_aug^T rows 0:DH = z^T, row DH = l^T).
  - v^T projections of a couple are column-packed into one psum tile.
  - z psum columns complete PROGRESSIVELY (column block [128i, 128(i+1))
    is final after strip i), so zT extraction, l/recip, and the
    out-projection are pipelined per strip-pair WITHIN each pair; out
    matmuls of strip-couple j are deferred one step and emitted between
    the next strip-pair's score and z matmuls as PE gap fillers.
  - output is written bf16 (host casts to f32): halves the dominant DMA
    stream.  psum->sbuf out-copies (scaled by 1/l) round-robin across
    VectorE / ScalarE / GpSimd.
"""

import os
import numpy as np
import ml_dtypes
from contextlib import ExitStack

B, S, H, DM, DH = 4, 1024, 12, 768, 64
N_CORES = 8
PAIRS = B * H
PPC = PAIRS // N_CORES      # pairs per core
CPC = PPC // 2              # couples per core

BF16 = ml_dtypes.bfloat16

LAST_EXEC_TIME_NS = None
LAST_RESULTS = None


def _core_pair_map():
    """(b, h) for each (core, slot).  Couple g = (head g//2, batch-half g%2);
    core c owns couples 3c..3c+2, slot s -> couple 3c + s//2, e = s%2."""
    m = []
    for c in range(N_CORES):
        row = []
        for s in range(PPC):
            g = 3 * c + s // 2
            h, bh, e = g // 2, g % 2, s % 2
            row.append((2 * bh + e, h))
        m.append(row)
    return m


def _strip_blocks(i, s_len):
    """128-aligned score blocks for strip i: start at the diagonal."""
    v = 128 * i
    if v < 512:
        return [(v, 512), (512, s_len)]
    return [(v, s_len)]


def build_nc(n_couples=CPC, s_len=S, dm=DM, dh=DH):
    import concourse.bacc as bacc
    import concourse.tile as tile
    import concourse.mybir as mybir

    f32 = mybir.dt.float32
    bf16 = mybir.dt.bfloat16
    KC = dm // 128
    NSQ = s_len // 128
    MMB = 512

    nc = bacc.Bacc("TRN2", target_bir_lowering=False, debug=False)

    fp8 = mybir.dt.float8e4
    # partition-major inputs: cheap 2-D DMAs.  x ships BOTH as an fp8 hi+lo
    # split (the qk projection runs DoubleRow fp8: hi*hi + hi*lo + lo*hi,
    # MORE accurate than bf16 at 3/4 the PE cost) and as bf16 (the v
    # projection keeps bf16: DoubleRow cannot write a column-packed psum
    # base-64 dst, and unpacked fp8 would cost more than packed bf16).
    xt = nc.declare_dram_parameter("xt", [2 * n_couples, 128, KC * s_len], bf16, isOutput=False)
    xhl = nc.declare_dram_parameter("xhl", [2 * n_couples, 128, 2 * KC * s_len], fp8, isOutput=False)
    wqk8 = nc.declare_dram_parameter("wqk8", [n_couples, 128, 2 * KC * 2 * dh], fp8, isOutput=False)
    wov = nc.declare_dram_parameter("wov", [n_couples, 128, dm + KC * dh], bf16, isOutput=False)
    ident = nc.declare_dram_parameter("ident", [128, 128], bf16, isOutput=False)
    out = nc.declare_dram_parameter("out", [2 * n_couples, NSQ // 2, 128, 2 * dm], bf16, isOutput=True)
    DR = mybir.MatmulPerfMode.DoubleRow

    Exp = mybir.ActivationFunctionType.Exp
    Copy = mybir.ActivationFunctionType.Copy
    WVO = KC * 2 * dh  # column offset of wv within wqkv

    with ExitStack() as ctx:
        tc = ctx.enter_context(tile.TileContext(nc))

        xt_pool = ctx.enter_context(tc.tile_pool(name="xt", bufs=4))
        xhl_pool = ctx.enter_context(tc.tile_pool(name="xhl", bufs=4))
        wqk8_pool = ctx.enter_context(tc.tile_pool(name="wqk8", bufs=n_couples))
        wov_pool = ctx.enter_context(tc.tile_pool(name="wov", bufs=n_couples))
        const_pool = ctx.enter_context(tc.tile_pool(name="const", bufs=1))
        qkT_pool = ctx.enter_context(tc.tile_pool(name="qkT", bufs=3))
        swap_pool = ctx.enter_context(tc.tile_pool(name="swap", bufs=3))
        vT_pool = ctx.enter_context(tc.tile_pool(name="vT", bufs=2))
        vaug_pool = ctx.enter_context(tc.tile_pool(name="vaug", bufs=2))
        pstrip_pool = ctx.enter_context(tc.tile_pool(name="pstrip", bufs=10))
        zT_pool = ctx.enter_context(tc.tile_pool(name="zT", bufs=4))
        lf_pool = ctx.enter_context(tc.tile_pool(name="lf", bufs=4))
        recip_pool = ctx.enter_context(tc.tile_pool(name="recip", bufs=4))
        osb_pool = ctx.enter_context(tc.tile_pool(name="osb", bufs=8))

        # PSUM: ps2 = 2-bank accumulators (qk^T / packed v^T / z^T);
        # scps = score blocks + v transposes; ops = out chunks + l columns.
        ps2 = ctx.enter_context(tc.tile_pool(name="ps2", bufs=2, space="PSUM"))
        scps = ctx.enter_context(tc.tile_pool(name="scps", bufs=2, space="PSUM"))
        ops_pool = ctx.enter_context(tc.tile_pool(name="ops", bufs=2, space="PSUM"))

        ones_sb = const_pool.tile([1, 1], bf16, name="ones_sb")
        nc.vector.memset(ones_sb[:], 1.0)
        FSC = 1.0 / 64.0  # descale: fp8 weights ship pre-scaled by 64

        # ---- loads are issued just-in-time, one couple ahead, so the sync
        # queue never builds a backlog that delays later stores (the DMA
        # completion semaphores are cumulative per queue) ----
        wqkv_sbs, wo_sbs, x_sbs = [], [], []
        kh = KC // 2

        def issue_couple_loads(g, fine):
            w8_sb = wqk8_pool.tile([128, 2 * KC * 2 * dh], fp8, name=f"wqk8_{g}", tag="wqk8")
            nc.sync.dma_start(w8_sb[:], wqk8[g])
            wov_sb = wov_pool.tile([128, dm + KC * dh], bf16, name=f"wov_{g}", tag="wov")
            nc.sync.dma_start(wov_sb[:], wov[g])
            wqkv_sbs.append(w8_sb)
            wo_sbs.append(wov_sb)
            for e in (0, 1):
                p = 2 * g + e
                x8 = xhl_pool.tile([128, 2 * KC * s_len], fp8, name=f"x8_{p}", tag="x8")
                if fine:
                    # split so the chunk-pair-0 matmuls start immediately
                    # (hi and lo of chunks 0-1 first)
                    for off in (0, KC * s_len):
                        nc.sync.dma_start(x8[:, off:off + 2 * s_len],
                                          xhl[p, :, off:off + 2 * s_len])
                    for off in (0, KC * s_len):
                        nc.sync.dma_start(x8[:, off + 2 * s_len:off + KC * s_len],
                                          xhl[p, :, off + 2 * s_len:off + KC * s_len])
                else:
                    nc.sync.dma_start(x8[:, :KC * s_len], xhl[p, :, :KC * s_len])
                    nc.sync.dma_start(x8[:, KC * s_len:], xhl[p, :, KC * s_len:])
                xtile = xt_pool.tile([128, KC * s_len], bf16, name=f"x_{p}", tag="x")
                nc.sync.dma_start(xtile[:, :kh * s_len], xt[p, :, :kh * s_len])
                nc.sync.dma_start(xtile[:, kh * s_len:], xt[p, :, kh * s_len:])
                x_sbs.append((x8, xtile))

        issue_couple_loads(0, fine=True)
        # ident (transposes) is not needed until mid-couple-0: load after
        ident_sb = const_pool.tile([128, 128], bf16, name="ident_sb")
        nc.sync.dma_start(ident_sb[:], ident[:, :])
        if n_couples > 1:
            issue_couple_loads(1, fine=False)

        # engine round-robin for out-copies (psum readers: DVE/ScalarE only)
        OUT_ENGS = [nc.vector, nc.scalar, nc.vector, nc.scalar,
                    nc.vector, nc.scalar, nc.vector, nc.scalar]
        out_rr = [0]
        dma_rr = [0]

        pending_out = []
        final_split = [False]

        def emit_one_pending():
            if pending_out:
                pending_out.pop(0)()

        def make_out(p, j, zT_sb, recip_sb, wo_sb):
            """Out-projection for strip-couple (j, j+1), row-packed."""
            def emit():
                o_sb = osb_pool.tile([128, 2 * dm], bf16, name=f"osb_{p}_{j}", tag="osb")
                for c0 in range(0, dm, MMB):
                    c1 = min(c0 + MMB, dm)
                    o_tiles = []
                    for dj in (0, 1):
                        o_ps = ops_pool.tile([128, 512], f32, name=f"ops_{p}_{j + dj}_{c0}", tag="ops")
                        nc.tensor.matmul(
                            o_ps[:, 0:c1 - c0],
                            lhsT=zT_sb[64 * dj:64 * dj + dh,
                                       (j + dj) * 128:(j + dj + 1) * 128],
                            rhs=wo_sb[64 * dj:64 * dj + dh, c0:c1],
                            start=True, stop=True,
                        )
                        o_tiles.append(o_ps)
                    for dj in (0, 1):
                        dst = o_sb[:, dj * dm + c0:dj * dm + c1]
                        osrc = o_tiles[dj][:, 0:c1 - c0]
                        scal = recip_sb[:, j + dj:j + dj + 1]
                        eng = OUT_ENGS[out_rr[0] % len(OUT_ENGS)]
                        out_rr[0] += 1
                        if eng is nc.scalar:
                            nc.scalar.mul(dst, osrc, scal)
                        else:
                            eng.tensor_scalar_mul(dst, osrc, scal)
                if final_split[0]:
                    # tail: halve store latency across two queues
                    nc.sync.dma_start(out[p, j // 2][:, :dm], o_sb[:, :dm])
                    nc.gpsimd.dma_start(out[p, j // 2][:, dm:], o_sb[:, dm:])
                else:
                    nc.sync.dma_start(out[p, j // 2], o_sb[:])
                dma_rr[0] += 1
            return emit

        for g in range(n_couples):
            if g + 2 < n_couples:
                issue_couple_loads(g + 2, fine=False)
            p0, p1 = 2 * g, 2 * g + 1
            x0, x1 = x_sbs[p0], x_sbs[p1]
            w8_sb = wqkv_sbs[g]
            wov_sb = wo_sbs[g]
            wo_sb = wov_sb  # out-proj reads cols [0:dm]
            KC2 = KC // 2
            XLO = KC * s_len   # column offset of the lo part in x8
            WLO = KC * 2 * dh  # column offset of the lo part in wqk8
            # fp8 DoubleRow term order: (xh,wh), (xl,wh) share a stationary
            TERMS = [(0, 0), (1, 0), (0, 1)]  # (x part, w part)

            def dr_w(wi, c2):
                off = WLO if wi else 0
                return (w8_sb[:, off + 2 * c2 * 2 * dh:off + 2 * (c2 + 1) * 2 * dh]
                        .rearrange("p (two m) -> p two m", two=2))

            def dr_x(x8, xi, c2, n0, n1):
                off = XLO if xi else 0
                return (x8[:, off:off + KC * s_len]
                        .rearrange("p (c n) -> p c n", n=s_len)
                        [:, 2 * c2:2 * c2 + 2, n0:n1])

            # ---- qk^T projections (DoubleRow fp8 hi/lo) ----
            qk_pss = [ps2.tile([128, s_len], f32, name=f"qkps_{p}", tag="ps2")
                      for p in (p0, p1)]
            for c2 in range(KC2):
                for ti, (xi, wi) in enumerate(TERMS):
                    w_ap = dr_w(wi, c2)
                    for e, xpair in ((0, x0), (1, x1)):
                        for n0 in range(0, s_len, MMB):
                            n1 = min(n0 + MMB, s_len)
                            nc.tensor.matmul(
                                qk_pss[e][:, n0:n1],
                                lhsT=w_ap,
                                rhs=dr_x(xpair[0], xi, c2, n0, n1),
                                start=(c2 == 0 and ti == 0), stop=(c2 == KC2 - 1 and ti == 2),
                                perf_mode=DR,
                                skip_group_check=not (c2 == 0 and ti == 0 and e == 0),
                            )
            qkTs, swaps = [], []
            for e, p in ((0, p0), (1, p1)):
                qkT_sb = qkT_pool.tile([128, s_len], bf16, name=f"qkT_{p}", tag="qkT")
                swap_sb = swap_pool.tile([128, s_len], bf16, name=f"swap_{p}", tag="swap")
                for ci, n0 in enumerate(range(0, s_len, MMB)):
                    n1 = min(n0 + MMB, s_len)
                    # psum->sbuf descale+cast, split ScalarE/VectorE
                    if ci % 2 == 0:
                        nc.scalar.mul(qkT_sb[:, n0:n1], qk_pss[e][:, n0:n1], FSC)
                    else:
                        nc.vector.tensor_scalar_mul(qkT_sb[:, n0:n1], qk_pss[e][:, n0:n1], FSC)
                nc.gpsimd.dma_start(swap_sb[0:dh, :], qkT_sb[dh:2 * dh, :])
                nc.gpsimd.dma_start(swap_sb[dh:2 * dh, :], qkT_sb[0:dh, :])
                qkTs.append(qkT_sb)
                swaps.append(swap_sb)
                emit_one_pending()

            # ---- v^T projections (bf16), column-packed across the couple ----
            vt_ps = ps2.tile([128, s_len], f32, name=f"vtps_{g}", tag="ps2")
            for kc in range(KC):
                for n0 in range(0, s_len, MMB):
                    n1 = min(n0 + MMB, s_len)
                    for e, xpair in ((0, x0), (1, x1)):
                        nc.tensor.matmul(
                            vt_ps[64 * e:64 * e + dh, n0:n1],
                            lhsT=wov_sb[:, dm + kc * dh:dm + (kc + 1) * dh],
                            rhs=xpair[1][:, kc * s_len + n0:kc * s_len + n1],
                            start=(kc == 0), stop=(kc == KC - 1),
                            skip_group_check=True,
                        )
            vT_sb = vT_pool.tile([128, s_len], bf16, name=f"vT_{g}", tag="vT")
            # split so transposes of the first strips start early
            nc.vector.tensor_copy(vT_sb[:, 0:512], vt_ps[:, 0:512])
            nc.vector.tensor_copy(vT_sb[:, 512:], vt_ps[:, 512:])
            emit_one_pending()

            # bf16 transposes, interleaved across the couple (rows 0/64 pack)
            vtrs = [scps.tile([128, NSQ * dh], bf16, name=f"vtr_{2 * g + e}", tag="scps")
                    for e in (0, 1)]
            for t in range(NSQ):
                for e in (0, 1):
                    nc.tensor.transpose(
                        vtrs[e][:, t * dh:(t + 1) * dh],
                        vT_sb[64 * e:64 * e + dh, t * 128:(t + 1) * 128],
                        ident_sb[64 * e:64 * e + dh, 64 * e:64 * e + dh],
                    )
            vaugs = []
            for e, p in ((0, p0), (1, p1)):
                vaug_sb = vaug_pool.tile([128, NSQ * (dh + 1)], bf16, name=f"vaug_{p}", tag="vaug")
                if g == 0:
                    # ones columns persist across pool reuse; set once
                    nc.gpsimd.memset(vaug_sb[:], 1.0)
                nc.vector.tensor_copy(
                    vaug_sb[:].rearrange("p (n d) -> p n d", d=dh + 1)[:, :, 0:dh],
                    vtrs[e][:].rearrange("p (n d) -> p n d", d=dh),
                )
                vaugs.append(vaug_sb)

            # ---- phase B: the couple's two pairs INTERLEAVED per strip-pair
            # (doubles the independent PE work per iteration, so exp/mask
            # latencies and psum rotation hide under the other pair) ----
            z_pss, zT_sbs2, lf_sbs2, recip_sbs2 = {}, {}, {}, {}
            for e, p in ((0, p0), (1, p1)):
                z_pss[e] = ps2.tile([dh + 1, s_len], f32, name=f"zps_{p}", tag="ps2")
                zT_sbs2[e] = zT_pool.tile([128, s_len], bf16, name=f"zT_{p}", tag="zT")
                lf_sbs2[e] = lf_pool.tile([1, s_len], bf16, name=f"lf_{p}", tag="lf")
                recip_sbs2[e] = recip_pool.tile([128, NSQ], f32, name=f"recip_{p}", tag="recip")

            for i0 in range(0, NSQ, 2):
                blocks0 = _strip_blocks(i0, s_len)
                blocks1 = _strip_blocks(i0 + 1, s_len)
                nblk = max(len(blocks0), len(blocks1))
                sc_tiles = {}
                # row-packed score matmuls for BOTH pairs
                for e, p in ((0, p0), (1, p1)):
                    qkT_sb, swap_sb = qkTs[e], swaps[e]
                    for bi in range(nblk):
                        for di, i, blocks in ((0, i0, blocks0), (1, i0 + 1, blocks1)):
                            bj = bi - (nblk - len(blocks))
                            if bj < 0:
                                continue
                            a, b = blocks[bj]
                            sc_ps = scps.tile([128, 512], f32, name=f"sc_{p}_{i}_{a}", tag="scps")
                            if di == 0:
                                lhsT = swap_sb[0:dh, i * 128:(i + 1) * 128]
                                rhs = qkT_sb[0:dh, a:b]
                            else:
                                lhsT = qkT_sb[dh:2 * dh, i * 128:(i + 1) * 128]
                                rhs = swap_sb[dh:2 * dh, a:b]
                            nc.tensor.matmul(
                                sc_ps[:, 0:b - a], lhsT=lhsT, rhs=rhs,
                                start=True, stop=True,
                            )
                            sc_tiles[(e, i, a)] = sc_ps

                # PE gap fillers: deferred out-couples run here
                emit_one_pending()
                emit_one_pending()

                # exp (ScalarE), diag mask (GpSimd), z matmuls, extraction
                for e, p in ((0, p0), (1, p1)):
                    vaug_sb = vaugs[e]
                    z_ps = z_pss[e]
                    zT_sb, lf_sb, recip_sb = zT_sbs2[e], lf_sbs2[e], recip_sbs2[e]
                    for di, i, blocks in ((0, i0, blocks0), (1, i0 + 1, blocks1)):
                        for (a, b) in blocks:
                            sc_ps = sc_tiles[(e, i, a)]
                            pt_sb = pstrip_pool.tile([128, 512], bf16, name=f"pt_{p}_{i}_{a}", tag="pstrip")
                            nc.scalar.activation(pt_sb[:, 0:b - a], sc_ps[:, 0:b - a], Exp)
                            if a == 128 * i:  # leading block holds the diag triangle
                                dst = pt_sb[:, 0:128]
                                nc.gpsimd.affine_select(
                                    out=dst, in_=dst,
                                    compare_op=mybir.AluOpType.is_ge,
                                    fill=0.0, base=0,
                                    pattern=[[1, 128]], channel_multiplier=-1,
                                )
                            nc.tensor.matmul(
                                z_ps[:, a:b],
                                lhsT=vaug_sb[:, i * (dh + 1):(i + 1) * (dh + 1)],
                                rhs=pt_sb[:, 0:b - a],
                                start=(i == 0), stop=(i == (b - 1) // 128),
                                skip_group_check=True,
                            )

                    # eager extraction: z cols [128*i0, 128*i0+256) are final
                    c0, c1 = 128 * i0, 128 * (i0 + 2)
                    nc.vector.tensor_copy(zT_sb[0:dh, c0:c1], z_ps[0:dh, c0:c1])
                    nc.gpsimd.dma_start(zT_sb[dh:2 * dh, c0:c1], zT_sb[0:dh, c0:c1])
                    nc.vector.tensor_copy(lf_sb[:, c0:c1], z_ps[dh:dh + 1, c0:c1])
                    l_ps = ops_pool.tile([128, 2], f32, name=f"lps_{p}_{i0}", tag="ops")
                    for dj in (0, 1):
                        nc.tensor.matmul(
                            l_ps[:, dj:dj + 1],
                            lhsT=lf_sb[:, (i0 + dj) * 128:(i0 + dj + 1) * 128],
                            rhs=ones_sb[:, :],
                            start=True, stop=True,
                        )
                    nc.vector.reciprocal(recip_sb[:, i0:i0 + 2], l_ps[:])
                    pending_out.append(make_out(p, i0, zT_sb, recip_sb, wo_sb))

        final_split[0] = True
        while pending_out:
            emit_one_pending()

    nc.finalize()
    _dedup_ldweights(nc, mybir)
    return nc


def _dedup_ldweights(nc, mybir):
    """Remove back-to-back duplicate Ldweights on the PE stream.

    bacc lowers every matmul to an Ldweights+Matmult pair and walrus runs
    with --enable-ldw-opt=false, so consecutive matmuls sharing a stationary
    operand reload it (~107 ns each).  Emission order makes same-weight
    matmuls adjacent; drop an Ldweights when it exactly repeats the previous
    one on the PE stream and carries no semaphore waits/updates."""
    pe = mybir.EngineType.PE
    removed = 0

    def footprint(inst):
        pos = getattr(inst, "tile_position", None) or (0, 0)
        size = getattr(inst, "tile_size", None) or (128, 128)
        return (pos[0], pos[0] + size[0], pos[1], pos[1] + size[1])

    def overlaps(a, b):
        return a[0] < b[1] and b[0] < a[1] and a[2] < b[3] and b[2] < a[3]

    for fn in nc.m.functions:
        for blk in fn.blocks:
            last = {}  # (pos, size) -> (sig, footprint)
            keep = []
            for inst in blk.instructions:
                if getattr(inst, "engine", None) == pe:
                    if isinstance(inst, mybir.InstLdweights):
                        key = (
                            repr(getattr(inst, "tile_position", None)),
                            repr(getattr(inst, "tile_size", None)),
                        )
                        sig = (
                            repr(inst.ins), repr(inst.perf_mode),
                            repr(inst.is_transpose),
                        )
                        si = inst.sync_info
                        syncfree = si is None or (not si.on_wait and not si.on_update)
                        prev = last.get(key)
                        fp = footprint(inst)
                        if prev is not None and prev[0] == sig and syncfree:
                            removed += 1
                            continue
                        # a new load invalidates any tracked load whose
                        # quadrant footprint it overwrites
                        for k in list(last):
                            if k != key and overlaps(last[k][1], fp):
                                del last[k]
                        last[key] = (sig, fp)
                    elif not isinstance(inst, mybir.InstMatmult):
                        last = {}
                keep.append(inst)
            if removed:
                del blk.instructions[:]
                for inst in keep:
                    blk.instructions.append(inst)
    return removed


E4M3 = ml_dtypes.float8_e4m3fn


def _split_fp8(a):
    """hi + lo fp8 decomposition: hi = e4m3(a), lo = e4m3(a - hi)."""
    hi = np.asarray(a, dtype=np.float32).astype(E4M3)
    lo = (np.asarray(a, dtype=np.float32) - hi.astype(np.float32)).astype(E4M3)
    return hi, lo


def prepare_shards(normalized_resid_pre, W_Q, b_Q, W_K, b_K, W_V, b_V, W_O, b_O):
    """Host-side layout: returns in_maps for the 8 cores."""
    x = np.asarray(normalized_resid_pre, dtype=np.float32)
    scale = 1.0 / np.sqrt(DH)
    KC = DM // 128

    pair_map = _core_pair_map()

    # x^T per (core, slot), partition-major: [128, KC*S]
    xt_f = x.transpose(0, 2, 3, 1)  # [B, H, DM, S]
    # qk weights pre-scaled by 64 for the fp8 split (descaled on device);
    # W_Q additionally folds 1/sqrt(DH) so scores come out pre-scaled
    wqk_h = 64.0 * np.concatenate(
        [np.asarray(W_Q) * scale, np.asarray(W_K)], axis=-1)  # [H, DM, 2DH]
    wv_h = np.asarray(W_V)  # [H, DM, DH]
    wo_h = np.asarray(W_O)  # [H, DH, DM]

    ident = np.eye(128).astype(BF16)

    in_maps = []
    for c in range(N_CORES):
        xts, wqks, wovs = [], [], []
        for s in range(PPC):
            b, h = pair_map[c][s]
            xts.append(
                xt_f[b, h].reshape(KC, 128, S).transpose(1, 0, 2).reshape(128, KC * S))
            if s % 2 == 0:
                wqks.append(
                    wqk_h[h].reshape(KC, 128, 2 * DH).transpose(1, 0, 2).reshape(128, KC * 2 * DH))
                wovs.append(np.concatenate(
                    [np.concatenate([wo_h[h], wo_h[h]], axis=0),
                     wv_h[h].reshape(KC, 128, DH).transpose(1, 0, 2).reshape(128, KC * DH)],
                    axis=1))
        xt_all = np.ascontiguousarray(np.stack(xts))
        xh_all, xl_all = _split_fp8(xt_all)
        w_all = np.ascontiguousarray(np.stack(wqks))
        wh_all, wl_all = _split_fp8(w_all)
        in_maps.append({
            "xt": xt_all.astype(BF16),
            "xhl": np.ascontiguousarray(
                np.concatenate([xh_all, xl_all], axis=2)),
            "wqk8": np.ascontiguousarray(
                np.concatenate([wh_all, wl_all], axis=2)),
            "wov": np.ascontiguousarray(np.stack(wovs)).astype(BF16),
            "ident": ident,
        })
    return in_maps


def _ensure_profile_hook():
    """The agent image lacks ``antenv.axon_hooks``; shim it and install the
    ctypes NTFF hook from trn_boot so trace=True works under axon."""
    import importlib
    import sys
    import types
    try:
        importlib.import_module("antenv.axon_hooks")
        return True
    except ImportError:
        pass
    try:
        import antenv
        mod = types.ModuleType("antenv.axon_hooks")
        _state = {"hook": None}
        mod.set_axon_ntff_profile_hook = lambda h: _state.__setitem__("hook", h)
        mod.get_axon_ntff_profile_hook = lambda: _state["hook"]
        sys.modules["antenv.axon_hooks"] = mod
        antenv.axon_hooks = mod
        from trn_agent_boot.trn_boot import _ntff_profile_via_ctypes
        hook = _ntff_profile_via_ctypes("/opt/axon/libaxon_pjrt.so")
        if hook is not None:
            mod.set_axon_ntff_profile_hook(hook)
        return hook is not None
    except Exception:
        return False


def kernel(**inputs):
    global LAST_EXEC_TIME_NS, LAST_RESULTS
    from concourse.bass_utils import run_bass_kernel_spmd

    in_maps = prepare_shards(**inputs)
    nc = build_nc()

    trace = bool(int(os.environ.get("KERNEL_PROFILE", "0")))
    tmpdir = None
    if trace:
        trace = _ensure_profile_hook()
        if trace:
            tmpdir = os.environ.get("KERNEL_PROFILE_DIR") or None
    res = run_bass_kernel_spmd(nc, in_maps, list(range(N_CORES)), trace=trace,
                               tmpdir=tmpdir)
    LAST_EXEC_TIME_NS = res.exec_time_ns
    LAST_RESULTS = res

    pair_map = _core_pair_map()
    out = np.empty((B, S, H, DM), dtype=np.float32)
    for c in range(N_CORES):
        dev = np.asarray(res.results[c]["out"], dtype=np.float32)
        # [PPC, S//256, 128, 2*DM] -> [PPC, S, DM]
        dev = (dev.reshape(PPC, S // 256, 128, 2, DM)
               .transpose(0, 1, 3, 2, 4).reshape(PPC, S, DM))
        for s in range(PPC):
            b, h = pair_map[c][s]
            out[b, :, h, :] = dev[s]

    b_O = np.asarray(inputs["b_O"], dtype=np.float32)
    b_V = np.asarray(inputs["b_V"], dtype=np.float32)
    b_Q = np.asarray(inputs["b_Q"], dtype=np.float32)
    b_K = np.asarray(inputs["b_K"], dtype=np.float32)
    if np.any(b_Q) or np.any(b_K):
        raise NotImplementedError("nonzero b_Q/b_K not supported by this kernel")
    extra = b_O[None, :] / H  # [1, DM] broadcast over heads
    if np.any(b_V):
        extra = extra + np.einsum(
            "hd,hdm->hm", b_V, np.asarray(inputs["W_O"], dtype=np.float32))
    if np.any(extra):
        out = out + extra[None, None]
    return np.ascontiguousarray(out, dtype=np.float32)


# revision 55
# speedup vs baseline: 1.0250x; 1.0250x over previous
"""Trainium2 Bass kernel for per-head attention (TransformerLens-style).

Reference computation (per batch b, head h, with x = resid[b, :, h, :]):
    q = x @ W_Q[h] + b_Q[h];  k = x @ W_K[h] + b_K[h];  v = x @ W_V[h] + b_V[h]
    scores = q @ k.T / sqrt(DH), causal-masked, softmax over keys
    z = P @ v;  out[b, :, h, :] = z @ W_O[h] + b_O / H

Shapes: B=4, S=1024, H=12, DM=768, DH=64.  B*H = 48 independent attention
problems; 8 NeuronCores get 6 each (pure data parallel, no collectives).

Device-side formulation:
  - pairs are grouped into COUPLES sharing a head: couple = (h, batch-half),
    so weights load once per couple and the qk-projection shares one
    Ldweights across both pairs' moving blocks.
  - host passes x^T (DM-major) in bf16; weights bf16, W_Q pre-scaled by
    1/sqrt(DH).  qk^T projection -> psum [128, S] (rows 0:64 q^T, 64:128
    k^T); a partition-swapped copy (swap_sb) lets consecutive score strips
    run ROW-PACKED (PE row groups 0/64 concurrently).
  - scores are computed TRANSPOSED (s^T[sk, sq]) with 128-aligned block
    starts (no wasted pre-diagonal columns); softmax exp runs on ScalarE;
    the diagonal-block causal mask is a bf16 0/1 multiply split between
    VectorE and GpSimd.  Row sums over sk come free via a ones column
    augmenting v (z_aug^T rows 0:DH = z^T, row DH = l^T).
  - v^T projections of a couple are column-packed into one psum tile.
  - z psum columns complete PROGRESSIVELY (column block [128i, 128(i+1))
    is final after strip i), so zT extraction, l/recip, and the
    out-projection are pipelined per strip-pair WITHIN each pair; out
    matmuls of strip-couple j are deferred one step and emitted between
    the next strip-pair's score and z matmuls as PE gap fillers.
  - output is written bf16 (host casts to f32): halves the dominant DMA
    stream.  psum->sbuf out-copies (scaled by 1/l) round-robin across
    VectorE / ScalarE / GpSimd.
"""

import os
import numpy as np
import ml_dtypes
from contextlib import ExitStack

B, S, H, DM, DH = 4, 1024, 12, 768, 64
N_CORES = 8
PAIRS = B * H
PPC = PAIRS // N_CORES      # pairs per core
CPC = PPC // 2              # couples per core

BF16 = ml_dtypes.bfloat16

LAST_EXEC_TIME_NS = None
LAST_RESULTS = None


def _core_pair_map():
    """(b, h) for each (core, slot).  Couple g = (head g//2, batch-half g%2);
    core c owns couples 3c..3c+2, slot s -> couple 3c + s//2, e = s%2."""
    m = []
    for c in range(N_CORES):
        row = []
        for s in range(PPC):
            g = 3 * c + s // 2
            h, bh, e = g // 2, g % 2, s % 2
            row.append((2 * bh + e, h))
        m.append(row)
    return m


def _strip_blocks(i, s_len):
    """128-aligned score blocks for strip i: start at the diagonal."""
    v = 128 * i
    if v < 512:
        return [(v, 512), (512, s_len)]
    return [(v, s_len)]


def build_nc(n_couples=CPC, s_len=S, dm=DM, dh=DH):
    import concourse.bacc as bacc
    import concourse.tile as tile
    import concourse.mybir as mybir

    f32 = mybir.dt.float32
    bf16 = mybir.dt.bfloat16
    KC = dm // 128
    NSQ = s_len // 128
    MMB = 512

    nc = bacc.Bacc("TRN2", target_bir_lowering=False, debug=False)

    # partition-major inputs: cheap 2-D DMAs.  (fp8 DoubleRow was evaluated
    # and measured on HW: DR matmuls stream 1 output column/cycle, same as
    # bf16, so a hi+lo fp8 split costs 1.5x bf16 — all-bf16 is optimal here.)
    xt = nc.declare_dram_parameter("xt", [2 * n_couples, 128, KC * s_len], bf16, isOutput=False)
    wqkv = nc.declare_dram_parameter("wqkv", [n_couples, 128, KC * (2 * dh + dh)], bf16, isOutput=False)
    wo = nc.declare_dram_parameter("wo", [n_couples, 128, dm], bf16, isOutput=False)
    ident = nc.declare_dram_parameter("ident", [128, 128], bf16, isOutput=False)
    out = nc.declare_dram_parameter("out", [2 * n_couples, NSQ // 2, 128, 2 * dm], bf16, isOutput=True)

    Exp = mybir.ActivationFunctionType.Exp
    Copy = mybir.ActivationFunctionType.Copy
    WVO = KC * 2 * dh  # column offset of wv within wqkv

    with ExitStack() as ctx:
        tc = ctx.enter_context(tile.TileContext(nc))

        xt_pool = ctx.enter_context(tc.tile_pool(name="xt", bufs=2 * n_couples))
        wqkv_pool = ctx.enter_context(tc.tile_pool(name="wqkv", bufs=n_couples))
        wo_pool = ctx.enter_context(tc.tile_pool(name="wo", bufs=n_couples))
        const_pool = ctx.enter_context(tc.tile_pool(name="const", bufs=1))
        qkT_pool = ctx.enter_context(tc.tile_pool(name="qkT", bufs=3))
        swap_pool = ctx.enter_context(tc.tile_pool(name="swap", bufs=3))
        vT_pool = ctx.enter_context(tc.tile_pool(name="vT", bufs=2))
        vaug_pool = ctx.enter_context(tc.tile_pool(name="vaug", bufs=2))
        pstrip_pool = ctx.enter_context(tc.tile_pool(name="pstrip", bufs=10))
        zT_pool = ctx.enter_context(tc.tile_pool(name="zT", bufs=4))
        lf_pool = ctx.enter_context(tc.tile_pool(name="lf", bufs=4))
        recip_pool = ctx.enter_context(tc.tile_pool(name="recip", bufs=4))
        osb_pool = ctx.enter_context(tc.tile_pool(name="osb", bufs=8))

        # PSUM: ps2 = 2-bank accumulators (qk^T / packed v^T / z^T);
        # scps = score blocks + v transposes; ops = out chunks + l columns.
        ps2 = ctx.enter_context(tc.tile_pool(name="ps2", bufs=2, space="PSUM"))
        scps = ctx.enter_context(tc.tile_pool(name="scps", bufs=2, space="PSUM"))
        ops_pool = ctx.enter_context(tc.tile_pool(name="ops", bufs=2, space="PSUM"))

        ones_sb = const_pool.tile([1, 1], bf16, name="ones_sb")
        nc.vector.memset(ones_sb[:], 1.0)

        # ---- loads are issued just-in-time, one couple ahead, so the sync
        # queue never builds a backlog that delays later stores (the DMA
        # completion semaphores are cumulative per queue) ----
        wqkv_sbs, wo_sbs, x_sbs = [], [], []
        kh = KC // 2

        def issue_couple_loads(g, fine):
            wqkv_sb = wqkv_pool.tile([128, KC * 3 * dh], bf16, name=f"wqkv_{g}", tag="wqkv")
            wo_sb = wo_pool.tile([128, dm], bf16, name=f"wo_{g}", tag="wo")
            if fine:
                # first couple: chunk-0 pieces first so matmuls start early
                nc.sync.dma_start(wqkv_sb[:, :2 * 2 * dh], wqkv[g, :, :2 * 2 * dh])
                xts = []
                for e in (0, 1):
                    p = 2 * g + e
                    xtile = xt_pool.tile([128, KC * s_len], bf16, name=f"x_{p}", tag="x")
                    nc.sync.dma_start(xtile[:, :512], xt[p, :, :512])
                    xts.append(xtile)
                    x_sbs.append(xtile)
                nc.sync.dma_start(wqkv_sb[:, 2 * 2 * dh:], wqkv[g, :, 2 * 2 * dh:])
                for e in (0, 1):
                    nc.sync.dma_start(xts[e][:, 512:kh * s_len], xt[2 * g + e, :, 512:kh * s_len])
                for e in (0, 1):
                    nc.sync.dma_start(xts[e][:, kh * s_len:], xt[2 * g + e, :, kh * s_len:])
                nc.sync.dma_start(wo_sb[:], wo[g])
            else:
                nc.sync.dma_start(wqkv_sb[:], wqkv[g])
                for e in (0, 1):
                    p = 2 * g + e
                    xtile = xt_pool.tile([128, KC * s_len], bf16, name=f"x_{p}", tag="x")
                    nc.sync.dma_start(xtile[:, :kh * s_len], xt[p, :, :kh * s_len])
                    nc.sync.dma_start(xtile[:, kh * s_len:], xt[p, :, kh * s_len:])
                    x_sbs.append(xtile)
                nc.sync.dma_start(wo_sb[:], wo[g])
            wqkv_sbs.append(wqkv_sb)
            wo_sbs.append(wo_sb)

        issue_couple_loads(0, fine=True)
        # ident (transposes) is not needed until mid-couple-0: load after
        ident_sb = const_pool.tile([128, 128], bf16, name="ident_sb")
        nc.sync.dma_start(ident_sb[:], ident[:, :])
        if n_couples > 1:
            issue_couple_loads(1, fine=False)

        # engine round-robin for out-copies (psum readers: DVE/ScalarE only)
        OUT_ENGS = [nc.vector, nc.scalar, nc.vector, nc.scalar,
                    nc.vector, nc.scalar, nc.vector, nc.scalar]
        out_rr = [0]
        dma_rr = [0]

        pending_out = []
        final_split = [False]

        def emit_one_pending():
            if pending_out:
                pending_out.pop(0)()

        def make_out(p, j, zT_sb, recip_sb, wo_sb):
            """Out-projection for strip-couple (j, j+1), row-packed."""
            def emit():
                o_sb = osb_pool.tile([128, 2 * dm], bf16, name=f"osb_{p}_{j}", tag="osb")
                for c0 in range(0, dm, MMB):
                    c1 = min(c0 + MMB, dm)
                    o_tiles = []
                    for dj in (0, 1):
                        o_ps = ops_pool.tile([128, 512], f32, name=f"ops_{p}_{j + dj}_{c0}", tag="ops")
                        nc.tensor.matmul(
                            o_ps[:, 0:c1 - c0],
                            lhsT=zT_sb[64 * dj:64 * dj + dh,
                                       (j + dj) * 128:(j + dj + 1) * 128],
                            rhs=wo_sb[64 * dj:64 * dj + dh, c0:c1],
                            start=True, stop=True,
                        )
                        o_tiles.append(o_ps)
                    for dj in (0, 1):
                        dst = o_sb[:, dj * dm + c0:dj * dm + c1]
                        osrc = o_tiles[dj][:, 0:c1 - c0]
                        scal = recip_sb[:, j + dj:j + dj + 1]
                        eng = OUT_ENGS[out_rr[0] % len(OUT_ENGS)]
                        out_rr[0] += 1
                        if eng is nc.scalar:
                            nc.scalar.mul(dst, osrc, scal)
                        else:
                            eng.tensor_scalar_mul(dst, osrc, scal)
                if final_split[0]:
                    # tail: halve store latency across two queues
                    nc.sync.dma_start(out[p, j // 2][:, :dm], o_sb[:, :dm])
                    nc.gpsimd.dma_start(out[p, j // 2][:, dm:], o_sb[:, dm:])
                else:
                    nc.sync.dma_start(out[p, j // 2], o_sb[:])
                dma_rr[0] += 1
            return emit

        for g in range(n_couples):
            if g + 2 < n_couples:
                issue_couple_loads(g + 2, fine=False)
            p0, p1 = 2 * g, 2 * g + 1
            x0, x1 = x_sbs[p0], x_sbs[p1]
            wqkv_sb = wqkv_sbs[g]
            wo_sb = wo_sbs[g]

            # ---- qk^T projections, shared stationary across the couple ----
            qk_pss = [ps2.tile([128, s_len], f32, name=f"qkps_{p}", tag="ps2")
                      for p in (p0, p1)]
            for kc in range(KC):
                for e, xtile in ((0, x0), (1, x1)):
                    for n0 in range(0, s_len, MMB):
                        n1 = min(n0 + MMB, s_len)
                        nc.tensor.matmul(
                            qk_pss[e][:, n0:n1],
                            lhsT=wqkv_sb[:, kc * 2 * dh:(kc + 1) * 2 * dh],
                            rhs=xtile[:, kc * s_len + n0:kc * s_len + n1],
                            start=(kc == 0), stop=(kc == KC - 1),
                            skip_group_check=(e == 1),
                        )
            qkTs, swaps = [], []
            for e, p in ((0, p0), (1, p1)):
                qkT_sb = qkT_pool.tile([128, s_len], bf16, name=f"qkT_{p}", tag="qkT")
                swap_sb = swap_pool.tile([128, s_len], bf16, name=f"swap_{p}", tag="swap")
                for ci, n0 in enumerate(range(0, s_len, MMB)):
                    n1 = min(n0 + MMB, s_len)
                    # psum->sbuf cast, split ScalarE/VectorE
                    if ci % 2 == 0:
                        nc.scalar.copy(qkT_sb[:, n0:n1], qk_pss[e][:, n0:n1])
                    else:
                        nc.vector.tensor_copy(qkT_sb[:, n0:n1], qk_pss[e][:, n0:n1])
                nc.gpsimd.dma_start(swap_sb[0:dh, :], qkT_sb[dh:2 * dh, :])
                nc.gpsimd.dma_start(swap_sb[dh:2 * dh, :], qkT_sb[0:dh, :])
                qkTs.append(qkT_sb)
                swaps.append(swap_sb)
                emit_one_pending()

            # ---- v^T projections, column-packed across the couple ----
            vt_ps = ps2.tile([128, s_len], f32, name=f"vtps_{g}", tag="ps2")
            for kc in range(KC):
                for n0 in range(0, s_len, MMB):
                    n1 = min(n0 + MMB, s_len)
                    for e, xtile in ((0, x0), (1, x1)):
                        nc.tensor.matmul(
                            vt_ps[64 * e:64 * e + dh, n0:n1],
                            lhsT=wqkv_sb[:, WVO + kc * dh:WVO + (kc + 1) * dh],
                            rhs=xtile[:, kc * s_len + n0:kc * s_len + n1],
                            start=(kc == 0), stop=(kc == KC - 1),
                            skip_group_check=True,
                        )
            vT_sb = vT_pool.tile([128, s_len], bf16, name=f"vT_{g}", tag="vT")
            # split so transposes of the first strips start early
            nc.vector.tensor_copy(vT_sb[:, 0:512], vt_ps[:, 0:512])
            nc.vector.tensor_copy(vT_sb[:, 512:], vt_ps[:, 512:])
            emit_one_pending()

            # bf16 transposes, interleaved across the couple (rows 0/64 pack)
            vtrs = [scps.tile([128, NSQ * dh], bf16, name=f"vtr_{2 * g + e}", tag="scps")
                    for e in (0, 1)]
            for t in range(NSQ):
                for e in (0, 1):
                    nc.tensor.transpose(
                        vtrs[e][:, t * dh:(t + 1) * dh],
                        vT_sb[64 * e:64 * e + dh, t * 128:(t + 1) * 128],
                        ident_sb[64 * e:64 * e + dh, 64 * e:64 * e + dh],
                    )
            vaugs = []
            for e, p in ((0, p0), (1, p1)):
                vaug_sb = vaug_pool.tile([128, NSQ * (dh + 1)], bf16, name=f"vaug_{p}", tag="vaug")
                if g == 0:
                    # ones columns persist across pool reuse; set once
                    nc.gpsimd.memset(vaug_sb[:], 1.0)
                nc.vector.tensor_copy(
                    vaug_sb[:].rearrange("p (n d) -> p n d", d=dh + 1)[:, :, 0:dh],
                    vtrs[e][:].rearrange("p (n d) -> p n d", d=dh),
                )
                vaugs.append(vaug_sb)

            # ---- phase B: the couple's two pairs INTERLEAVED per strip-pair
            # (doubles the independent PE work per iteration, so exp/mask
            # latencies and psum rotation hide under the other pair) ----
            z_pss, zT_sbs2, lf_sbs2, recip_sbs2 = {}, {}, {}, {}
            for e, p in ((0, p0), (1, p1)):
                z_pss[e] = ps2.tile([dh + 1, s_len], f32, name=f"zps_{p}", tag="ps2")
                zT_sbs2[e] = zT_pool.tile([128, s_len], bf16, name=f"zT_{p}", tag="zT")
                lf_sbs2[e] = lf_pool.tile([1, s_len], bf16, name=f"lf_{p}", tag="lf")
                recip_sbs2[e] = recip_pool.tile([128, NSQ], f32, name=f"recip_{p}", tag="recip")

            for i0 in range(0, NSQ, 2):
                blocks0 = _strip_blocks(i0, s_len)
                blocks1 = _strip_blocks(i0 + 1, s_len)
                nblk = max(len(blocks0), len(blocks1))
                sc_tiles = {}
                # row-packed score matmuls for BOTH pairs
                for e, p in ((0, p0), (1, p1)):
                    qkT_sb, swap_sb = qkTs[e], swaps[e]
                    for bi in range(nblk):
                        for di, i, blocks in ((0, i0, blocks0), (1, i0 + 1, blocks1)):
                            bj = bi - (nblk - len(blocks))
                            if bj < 0:
                                continue
                            a, b = blocks[bj]
                            sc_ps = scps.tile([128, 512], f32, name=f"sc_{p}_{i}_{a}", tag="scps")
                            if di == 0:
                                lhsT = swap_sb[0:dh, i * 128:(i + 1) * 128]
                                rhs = qkT_sb[0:dh, a:b]
                            else:
                                lhsT = qkT_sb[dh:2 * dh, i * 128:(i + 1) * 128]
                                rhs = swap_sb[dh:2 * dh, a:b]
                            nc.tensor.matmul(
                                sc_ps[:, 0:b - a], lhsT=lhsT, rhs=rhs,
                                start=True, stop=True,
                            )
                            sc_tiles[(e, i, a)] = sc_ps

                # PE gap fillers: deferred out-couples run here
                emit_one_pending()
                emit_one_pending()

                # exp (ScalarE), diag mask (GpSimd), z matmuls, extraction
                for e, p in ((0, p0), (1, p1)):
                    vaug_sb = vaugs[e]
                    z_ps = z_pss[e]
                    zT_sb, lf_sb, recip_sb = zT_sbs2[e], lf_sbs2[e], recip_sbs2[e]
                    for di, i, blocks in ((0, i0, blocks0), (1, i0 + 1, blocks1)):
                        for (a, b) in blocks:
                            sc_ps = sc_tiles[(e, i, a)]
                            pt_sb = pstrip_pool.tile([128, 512], bf16, name=f"pt_{p}_{i}_{a}", tag="pstrip")
                            nc.scalar.activation(pt_sb[:, 0:b - a], sc_ps[:, 0:b - a], Exp)
                            if a == 128 * i:  # leading block holds the diag triangle
                                dst = pt_sb[:, 0:128]
                                nc.gpsimd.affine_select(
                                    out=dst, in_=dst,
                                    compare_op=mybir.AluOpType.is_ge,
                                    fill=0.0, base=0,
                                    pattern=[[1, 128]], channel_multiplier=-1,
                                )
                            nc.tensor.matmul(
                                z_ps[:, a:b],
                                lhsT=vaug_sb[:, i * (dh + 1):(i + 1) * (dh + 1)],
                                rhs=pt_sb[:, 0:b - a],
                                start=(i == 0), stop=(i == (b - 1) // 128),
                                skip_group_check=True,
                            )

                    # eager extraction: z cols [128*i0, 128*i0+256) are final
                    c0, c1 = 128 * i0, 128 * (i0 + 2)
                    nc.vector.tensor_copy(zT_sb[0:dh, c0:c1], z_ps[0:dh, c0:c1])
                    nc.gpsimd.dma_start(zT_sb[dh:2 * dh, c0:c1], zT_sb[0:dh, c0:c1])
                    nc.vector.tensor_copy(lf_sb[:, c0:c1], z_ps[dh:dh + 1, c0:c1])
                    l_ps = ops_pool.tile([128, 2], f32, name=f"lps_{p}_{i0}", tag="ops")
                    for dj in (0, 1):
                        nc.tensor.matmul(
                            l_ps[:, dj:dj + 1],
                            lhsT=lf_sb[:, (i0 + dj) * 128:(i0 + dj + 1) * 128],
                            rhs=ones_sb[:, :],
                            start=True, stop=True,
                        )
                    nc.vector.reciprocal(recip_sb[:, i0:i0 + 2], l_ps[:])
                    pending_out.append(make_out(p, i0, zT_sb, recip_sb, wo_sb))

        final_split[0] = True
        while pending_out:
            emit_one_pending()

    nc.finalize()
    _dedup_ldweights(nc, mybir)
    return nc


def _dedup_ldweights(nc, mybir):
    """Remove back-to-back duplicate Ldweights on the PE stream.

    bacc lowers every matmul to an Ldweights+Matmult pair and walrus runs
    with --enable-ldw-opt=false, so consecutive matmuls sharing a stationary
    operand reload it (~107 ns each).  Emission order makes same-weight
    matmuls adjacent; drop an Ldweights when it exactly repeats the previous
    one on the PE stream and carries no semaphore waits/updates."""
    pe = mybir.EngineType.PE
    removed = 0

    def footprint(inst):
        pos = getattr(inst, "tile_position", None) or (0, 0)
        size = getattr(inst, "tile_size", None) or (128, 128)
        return (pos[0], pos[0] + size[0], pos[1], pos[1] + size[1])

    def overlaps(a, b):
        return a[0] < b[1] and b[0] < a[1] and a[2] < b[3] and b[2] < a[3]

    for fn in nc.m.functions:
        for blk in fn.blocks:
            last = {}  # (pos, size) -> (sig, footprint)
            keep = []
            for inst in blk.instructions:
                if getattr(inst, "engine", None) == pe:
                    if isinstance(inst, mybir.InstLdweights):
                        key = (
                            repr(getattr(inst, "tile_position", None)),
                            repr(getattr(inst, "tile_size", None)),
                        )
                        sig = (
                            repr(inst.ins), repr(inst.perf_mode),
                            repr(inst.is_transpose),
                        )
                        si = inst.sync_info
                        syncfree = si is None or (not si.on_wait and not si.on_update)
                        prev = last.get(key)
                        fp = footprint(inst)
                        if prev is not None and prev[0] == sig and syncfree:
                            removed += 1
                            continue
                        # a new load invalidates any tracked load whose
                        # quadrant footprint it overwrites
                        for k in list(last):
                            if k != key and overlaps(last[k][1], fp):
                                del last[k]
                        last[key] = (sig, fp)
                    elif not isinstance(inst, mybir.InstMatmult):
                        last = {}
                keep.append(inst)
            if removed:
                del blk.instructions[:]
                for inst in keep:
                    blk.instructions.append(inst)
    return removed


E4M3 = ml_dtypes.float8_e4m3fn


def _split_fp8(a):
    """hi + lo fp8 decomposition: hi = e4m3(a), lo = e4m3(a - hi)."""
    hi = np.asarray(a, dtype=np.float32).astype(E4M3)
    lo = (np.asarray(a, dtype=np.float32) - hi.astype(np.float32)).astype(E4M3)
    return hi, lo


def prepare_shards(normalized_resid_pre, W_Q, b_Q, W_K, b_K, W_V, b_V, W_O, b_O):
    """Host-side layout: returns in_maps for the 8 cores."""
    x = np.asarray(normalized_resid_pre, dtype=np.float32)
    scale = 1.0 / np.sqrt(DH)
    KC = DM // 128

    pair_map = _core_pair_map()

    # x^T per (core, slot), partition-major: [128, KC*S]
    xt_f = x.transpose(0, 2, 3, 1)  # [B, H, DM, S]
    # W_Q pre-scaled by 1/sqrt(DH) so scores come out pre-scaled
    wqk_h = np.concatenate([np.asarray(W_Q) * scale, np.asarray(W_K)], axis=-1)
    wv_h = np.asarray(W_V)  # [H, DM, DH]
    wo_h = np.asarray(W_O)  # [H, DH, DM]

    ident = np.eye(128).astype(BF16)

    in_maps = []
    for c in range(N_CORES):
        xts, wqkvs, wos = [], [], []
        for s in range(PPC):
            b, h = pair_map[c][s]
            xts.append(
                xt_f[b, h].reshape(KC, 128, S).transpose(1, 0, 2).reshape(128, KC * S))
            if s % 2 == 0:
                wqkvs.append(np.concatenate(
                    [wqk_h[h].reshape(KC, 128, 2 * DH).transpose(1, 0, 2).reshape(128, KC * 2 * DH),
                     wv_h[h].reshape(KC, 128, DH).transpose(1, 0, 2).reshape(128, KC * DH)],
                    axis=1))
                wos.append(np.concatenate([wo_h[h], wo_h[h]], axis=0))  # [128, DM]
        in_maps.append({
            "xt": np.ascontiguousarray(np.stack(xts)).astype(BF16),
            "wqkv": np.ascontiguousarray(np.stack(wqkvs)).astype(BF16),
            "wo": np.ascontiguousarray(np.stack(wos)).astype(BF16),
            "ident": ident,
        })
    return in_maps


def _ensure_profile_hook():
    """The agent image lacks ``antenv.axon_hooks``; shim it and install the
    ctypes NTFF hook from trn_boot so trace=True works under axon."""
    import importlib
    import sys
    import types
    try:
        importlib.import_module("antenv.axon_hooks")
        return True
    except ImportError:
        pass
    try:
        import antenv
        mod = types.ModuleType("antenv.axon_hooks")
        _state = {"hook": None}
        mod.set_axon_ntff_profile_hook = lambda h: _state.__setitem__("hook", h)
        mod.get_axon_ntff_profile_hook = lambda: _state["hook"]
        sys.modules["antenv.axon_hooks"] = mod
        antenv.axon_hooks = mod
        from trn_agent_boot.trn_boot import _ntff_profile_via_ctypes
        hook = _ntff_profile_via_ctypes("/opt/axon/libaxon_pjrt.so")
        if hook is not None:
            mod.set_axon_ntff_profile_hook(hook)
        return hook is not None
    except Exception:
        return False


def kernel(**inputs):
    global LAST_EXEC_TIME_NS, LAST_RESULTS
    from concourse.bass_utils import run_bass_kernel_spmd

    in_maps = prepare_shards(**inputs)
    nc = build_nc()

    trace = bool(int(os.environ.get("KERNEL_PROFILE", "0")))
    tmpdir = None
    if trace:
        trace = _ensure_profile_hook()
        if trace:
            tmpdir = os.environ.get("KERNEL_PROFILE_DIR") or None
    res = run_bass_kernel_spmd(nc, in_maps, list(range(N_CORES)), trace=trace,
                               tmpdir=tmpdir)
    LAST_EXEC_TIME_NS = res.exec_time_ns
    LAST_RESULTS = res

    pair_map = _core_pair_map()
    out = np.empty((B, S, H, DM), dtype=np.float32)
    for c in range(N_CORES):
        dev = np.asarray(res.results[c]["out"], dtype=np.float32)
        # [PPC, S//256, 128, 2*DM] -> [PPC, S, DM]
        dev = (dev.reshape(PPC, S // 256, 128, 2, DM)
               .transpose(0, 1, 3, 2, 4).reshape(PPC, S, DM))
        for s in range(PPC):
            b, h = pair_map[c][s]
            out[b, :, h, :] = dev[s]

    b_O = np.asarray(inputs["b_O"], dtype=np.float32)
    b_V = np.asarray(inputs["b_V"], dtype=np.float32)
    b_Q = np.asarray(inputs["b_Q"], dtype=np.float32)
    b_K = np.asarray(inputs["b_K"], dtype=np.float32)
    if np.any(b_Q) or np.any(b_K):
        raise NotImplementedError("nonzero b_Q/b_K not supported by this kernel")
    extra = b_O[None, :] / H  # [1, DM] broadcast over heads
    if np.any(b_V):
        extra = extra + np.einsum(
            "hd,hdm->hm", b_V, np.asarray(inputs["W_O"], dtype=np.float32))
    if np.any(extra):
        out = out + extra[None, None]
    return np.ascontiguousarray(out, dtype=np.float32)


# revision 56
# speedup vs baseline: 1.0701x; 1.0440x over previous
"""Trainium2 Bass kernel for per-head attention (TransformerLens-style).

Reference computation (per batch b, head h, with x = resid[b, :, h, :]):
    q = x @ W_Q[h] + b_Q[h];  k = x @ W_K[h] + b_K[h];  v = x @ W_V[h] + b_V[h]
    scores = q @ k.T / sqrt(DH), causal-masked, softmax over keys
    z = P @ v;  out[b, :, h, :] = z @ W_O[h] + b_O / H

Shapes: B=4, S=1024, H=12, DM=768, DH=64.  B*H = 48 independent attention
problems; 8 NeuronCores get 6 each (pure data parallel, no collectives).

Device-side formulation:
  - pairs are grouped into COUPLES sharing a head: couple = (h, batch-half),
    so weights load once per couple and the qk-projection shares one
    Ldweights across both pairs' moving blocks.
  - host passes x^T (DM-major) in bf16; weights bf16, W_Q pre-scaled by
    1/sqrt(DH).  qk^T projection -> psum [128, S] (rows 0:64 q^T, 64:128
    k^T); a partition-swapped copy (swap_sb) lets consecutive score strips
    run ROW-PACKED (PE row groups 0/64 concurrently).
  - scores are computed TRANSPOSED (s^T[sk, sq]) with 128-aligned block
    starts (no wasted pre-diagonal columns); softmax exp runs on ScalarE;
    the diagonal-block causal mask is a bf16 0/1 multiply split between
    VectorE and GpSimd.  Row sums over sk come free via a ones column
    augmenting v (z_aug^T rows 0:DH = z^T, row DH = l^T).
  - v^T projections of a couple are column-packed into one psum tile.
  - z psum columns complete PROGRESSIVELY (column block [128i, 128(i+1))
    is final after strip i), so zT extraction, l/recip, and the
    out-projection are pipelined per strip-pair WITHIN each pair; out
    matmuls of strip-couple j are deferred one step and emitted between
    the next strip-pair's score and z matmuls as PE gap fillers.
  - output is written bf16 (host casts to f32): halves the dominant DMA
    stream.  psum->sbuf out-copies (scaled by 1/l) round-robin across
    VectorE / ScalarE / GpSimd.
"""

import os
import numpy as np
import ml_dtypes
from contextlib import ExitStack

B, S, H, DM, DH = 4, 1024, 12, 768, 64
N_CORES = 8
PAIRS = B * H
PPC = PAIRS // N_CORES      # pairs per core
CPC = PPC // 2              # couples per core

BF16 = ml_dtypes.bfloat16

LAST_EXEC_TIME_NS = None
LAST_RESULTS = None


def _core_pair_map():
    """(b, h) for each (core, slot).  Couple g = (head g//2, batch-half g%2);
    core c owns couples 3c..3c+2, slot s -> couple 3c + s//2, e = s%2."""
    m = []
    for c in range(N_CORES):
        row = []
        for s in range(PPC):
            g = 3 * c + s // 2
            h, bh, e = g // 2, g % 2, s % 2
            row.append((2 * bh + e, h))
        m.append(row)
    return m


def _strip_blocks(i, s_len):
    """128-aligned score blocks for strip i: start at the diagonal."""
    v = 128 * i
    if v < 512:
        return [(v, 512), (512, s_len)]
    return [(v, s_len)]


def build_nc(n_couples=CPC, s_len=S, dm=DM, dh=DH):
    import concourse.bacc as bacc
    import concourse.tile as tile
    import concourse.mybir as mybir

    f32 = mybir.dt.float32
    bf16 = mybir.dt.bfloat16
    KC = dm // 128
    NSQ = s_len // 128
    MMB = 512

    nc = bacc.Bacc("TRN2", target_bir_lowering=False, debug=False)

    # partition-major inputs: cheap 2-D DMAs.  (fp8 DoubleRow was evaluated
    # and measured on HW: DR matmuls stream 1 output column/cycle, same as
    # bf16, so a hi+lo fp8 split costs 1.5x bf16 — all-bf16 is optimal here.)
    xt = nc.declare_dram_parameter("xt", [2 * n_couples, 128, KC * s_len], bf16, isOutput=False)
    wqkv = nc.declare_dram_parameter("wqkv", [n_couples, 128, KC * (2 * dh + dh)], bf16, isOutput=False)
    wo = nc.declare_dram_parameter("wo", [n_couples, 128, dm], bf16, isOutput=False)
    ident = nc.declare_dram_parameter("ident", [128, 128], bf16, isOutput=False)
    out = nc.declare_dram_parameter("out", [2 * n_couples, NSQ // 2, 128, 2 * dm], bf16, isOutput=True)

    Exp = mybir.ActivationFunctionType.Exp
    Copy = mybir.ActivationFunctionType.Copy
    WVO = KC * 2 * dh  # column offset of wv within wqkv

    with ExitStack() as ctx:
        tc = ctx.enter_context(tile.TileContext(nc))

        xt_pool = ctx.enter_context(tc.tile_pool(name="xt", bufs=2 * n_couples))
        wqkv_pool = ctx.enter_context(tc.tile_pool(name="wqkv", bufs=n_couples))
        wo_pool = ctx.enter_context(tc.tile_pool(name="wo", bufs=n_couples))
        const_pool = ctx.enter_context(tc.tile_pool(name="const", bufs=1))
        qkT_pool = ctx.enter_context(tc.tile_pool(name="qkT", bufs=3))
        swap_pool = ctx.enter_context(tc.tile_pool(name="swap", bufs=3))
        vT_pool = ctx.enter_context(tc.tile_pool(name="vT", bufs=2))
        vaug_pool = ctx.enter_context(tc.tile_pool(name="vaug", bufs=2))
        pstrip_pool = ctx.enter_context(tc.tile_pool(name="pstrip", bufs=10))
        zT_pool = ctx.enter_context(tc.tile_pool(name="zT", bufs=4))
        lf_pool = ctx.enter_context(tc.tile_pool(name="lf", bufs=4))
        recip_pool = ctx.enter_context(tc.tile_pool(name="recip", bufs=4))
        osb_pool = ctx.enter_context(tc.tile_pool(name="osb", bufs=8))

        # PSUM: ps2 = 2-bank accumulators (qk^T / packed v^T / z^T);
        # scps = score blocks + v transposes; ops = out chunks + l columns.
        ps2 = ctx.enter_context(tc.tile_pool(name="ps2", bufs=2, space="PSUM"))
        scps = ctx.enter_context(tc.tile_pool(name="scps", bufs=2, space="PSUM"))
        ops_pool = ctx.enter_context(tc.tile_pool(name="ops", bufs=2, space="PSUM"))

        ones_sb = const_pool.tile([1, 1], bf16, name="ones_sb")
        nc.vector.memset(ones_sb[:], 1.0)

        # ---- loads are issued just-in-time, one couple ahead, so the sync
        # queue never builds a backlog that delays later stores (the DMA
        # completion semaphores are cumulative per queue) ----
        wqkv_sbs, wo_sbs, x_sbs = [], [], []
        kh = KC // 2

        def issue_couple_loads(g, fine):
            wqkv_sb = wqkv_pool.tile([128, KC * 3 * dh], bf16, name=f"wqkv_{g}", tag="wqkv")
            wo_sb = wo_pool.tile([128, dm], bf16, name=f"wo_{g}", tag="wo")
            if fine:
                # first couple: chunk-0 pieces first so matmuls start early
                nc.sync.dma_start(wqkv_sb[:, :2 * 2 * dh], wqkv[g, :, :2 * 2 * dh])
                xts = []
                for e in (0, 1):
                    p = 2 * g + e
                    xtile = xt_pool.tile([128, KC * s_len], bf16, name=f"x_{p}", tag="x")
                    nc.sync.dma_start(xtile[:, :512], xt[p, :, :512])
                    xts.append(xtile)
                    x_sbs.append(xtile)
                nc.sync.dma_start(wqkv_sb[:, 2 * 2 * dh:], wqkv[g, :, 2 * 2 * dh:])
                for e in (0, 1):
                    nc.sync.dma_start(xts[e][:, 512:kh * s_len], xt[2 * g + e, :, 512:kh * s_len])
                for e in (0, 1):
                    nc.sync.dma_start(xts[e][:, kh * s_len:], xt[2 * g + e, :, kh * s_len:])
                nc.sync.dma_start(wo_sb[:], wo[g])
            else:
                nc.sync.dma_start(wqkv_sb[:], wqkv[g])
                for e in (0, 1):
                    p = 2 * g + e
                    xtile = xt_pool.tile([128, KC * s_len], bf16, name=f"x_{p}", tag="x")
                    nc.sync.dma_start(xtile[:, :kh * s_len], xt[p, :, :kh * s_len])
                    nc.sync.dma_start(xtile[:, kh * s_len:], xt[p, :, kh * s_len:])
                    x_sbs.append(xtile)
                nc.sync.dma_start(wo_sb[:], wo[g])
            wqkv_sbs.append(wqkv_sb)
            wo_sbs.append(wo_sb)

        issue_couple_loads(0, fine=True)
        # ident (transposes) is not needed until mid-couple-0: load after
        ident_sb = const_pool.tile([128, 128], bf16, name="ident_sb")
        nc.sync.dma_start(ident_sb[:], ident[:, :])
        if n_couples > 1:
            issue_couple_loads(1, fine=False)

        # engine round-robin for out-copies (psum readers: DVE/ScalarE only)
        OUT_ENGS = [nc.vector, nc.scalar, nc.vector, nc.scalar,
                    nc.vector, nc.scalar, nc.vector, nc.scalar]
        out_rr = [0]
        dma_rr = [0]

        pending_out = []
        final_split = [False]

        def emit_one_pending():
            if pending_out:
                pending_out.pop(0)()

        def make_out(p, j, zT_sb, recip_sb, wo_sb):
            """Out-projection for strip-couple (j, j+1), row-packed."""
            def emit():
                o_sb = osb_pool.tile([128, 2 * dm], bf16, name=f"osb_{p}_{j}", tag="osb")
                for c0 in range(0, dm, MMB):
                    c1 = min(c0 + MMB, dm)
                    o_tiles = []
                    for dj in (0, 1):
                        o_ps = ops_pool.tile([128, 512], f32, name=f"ops_{p}_{j + dj}_{c0}", tag="ops")
                        nc.tensor.matmul(
                            o_ps[:, 0:c1 - c0],
                            lhsT=zT_sb[64 * dj:64 * dj + dh,
                                       (j + dj) * 128:(j + dj + 1) * 128],
                            rhs=wo_sb[64 * dj:64 * dj + dh, c0:c1],
                            start=True, stop=True,
                        )
                        o_tiles.append(o_ps)
                    for dj in (0, 1):
                        dst = o_sb[:, dj * dm + c0:dj * dm + c1]
                        osrc = o_tiles[dj][:, 0:c1 - c0]
                        scal = recip_sb[:, j + dj:j + dj + 1]
                        eng = OUT_ENGS[out_rr[0] % len(OUT_ENGS)]
                        out_rr[0] += 1
                        if eng is nc.scalar:
                            nc.scalar.mul(dst, osrc, scal)
                        else:
                            eng.tensor_scalar_mul(dst, osrc, scal)
                if final_split[0]:
                    # tail: halve store latency across two queues
                    nc.sync.dma_start(out[p, j // 2][:, :dm], o_sb[:, :dm])
                    nc.gpsimd.dma_start(out[p, j // 2][:, dm:], o_sb[:, dm:])
                else:
                    nc.sync.dma_start(out[p, j // 2], o_sb[:])
                dma_rr[0] += 1
            return emit

        for g in range(n_couples):
            if g + 2 < n_couples:
                issue_couple_loads(g + 2, fine=False)
            p0, p1 = 2 * g, 2 * g + 1
            x0, x1 = x_sbs[p0], x_sbs[p1]
            wqkv_sb = wqkv_sbs[g]
            wo_sb = wo_sbs[g]

            # ---- qk^T projections, shared stationary across the couple ----
            qk_pss = [ps2.tile([128, s_len], f32, name=f"qkps_{p}", tag="ps2")
                      for p in (p0, p1)]
            for kc in range(KC):
                for e, xtile in ((0, x0), (1, x1)):
                    for n0 in range(0, s_len, MMB):
                        n1 = min(n0 + MMB, s_len)
                        nc.tensor.matmul(
                            qk_pss[e][:, n0:n1],
                            lhsT=wqkv_sb[:, kc * 2 * dh:(kc + 1) * 2 * dh],
                            rhs=xtile[:, kc * s_len + n0:kc * s_len + n1],
                            start=(kc == 0), stop=(kc == KC - 1),
                            skip_group_check=(e == 1),
                        )
            qkTs, swaps = [], []
            for e, p in ((0, p0), (1, p1)):
                qkT_sb = qkT_pool.tile([128, s_len], bf16, name=f"qkT_{p}", tag="qkT")
                swap_sb = swap_pool.tile([128, s_len], bf16, name=f"swap_{p}", tag="swap")
                for ci, n0 in enumerate(range(0, s_len, MMB)):
                    n1 = min(n0 + MMB, s_len)
                    # psum->sbuf cast, split ScalarE/VectorE
                    if ci % 2 == 0:
                        nc.scalar.copy(qkT_sb[:, n0:n1], qk_pss[e][:, n0:n1])
                    else:
                        nc.vector.tensor_copy(qkT_sb[:, n0:n1], qk_pss[e][:, n0:n1])
                nc.gpsimd.dma_start(swap_sb[0:dh, :], qkT_sb[dh:2 * dh, :])
                nc.gpsimd.dma_start(swap_sb[dh:2 * dh, :], qkT_sb[0:dh, :])
                qkTs.append(qkT_sb)
                swaps.append(swap_sb)
                emit_one_pending()

            # ---- v^T projections, column-packed across the couple ----
            vt_ps = ps2.tile([128, s_len], f32, name=f"vtps_{g}", tag="ps2")
            for kc in range(KC):
                for n0 in range(0, s_len, MMB):
                    n1 = min(n0 + MMB, s_len)
                    for e, xtile in ((0, x0), (1, x1)):
                        nc.tensor.matmul(
                            vt_ps[64 * e:64 * e + dh, n0:n1],
                            lhsT=wqkv_sb[:, WVO + kc * dh:WVO + (kc + 1) * dh],
                            rhs=xtile[:, kc * s_len + n0:kc * s_len + n1],
                            start=(kc == 0), stop=(kc == KC - 1),
                            skip_group_check=True,
                        )
            vT_sb = vT_pool.tile([128, s_len], bf16, name=f"vT_{g}", tag="vT")
            # split so transposes of the first strips start early
            nc.vector.tensor_copy(vT_sb[:, 0:512], vt_ps[:, 0:512])
            nc.vector.tensor_copy(vT_sb[:, 512:], vt_ps[:, 512:])
            emit_one_pending()

            # bf16 transposes, interleaved across the couple (rows 0/64 pack)
            vtrs = [scps.tile([128, NSQ * dh], bf16, name=f"vtr_{2 * g + e}", tag="scps")
                    for e in (0, 1)]
            for t in range(NSQ):
                for e in (0, 1):
                    nc.tensor.transpose(
                        vtrs[e][:, t * dh:(t + 1) * dh],
                        vT_sb[64 * e:64 * e + dh, t * 128:(t + 1) * 128],
                        ident_sb[64 * e:64 * e + dh, 64 * e:64 * e + dh],
                    )
            vaugs = []
            for e, p in ((0, p0), (1, p1)):
                vaug_sb = vaug_pool.tile([128, NSQ * (dh + 1)], bf16, name=f"vaug_{p}", tag="vaug")
                if g == 0:
                    # ones columns persist across pool reuse; set once
                    nc.gpsimd.memset(vaug_sb[:], 1.0)
                nc.vector.tensor_copy(
                    vaug_sb[:].rearrange("p (n d) -> p n d", d=dh + 1)[:, :, 0:dh],
                    vtrs[e][:].rearrange("p (n d) -> p n d", d=dh),
                )
                vaugs.append(vaug_sb)

            # ---- phase B: the couple's two pairs INTERLEAVED per strip-pair
            # (doubles the independent PE work per iteration, so exp/mask
            # latencies and psum rotation hide under the other pair) ----
            z_pss, zT_sbs2, lf_sbs2, recip_sbs2 = {}, {}, {}, {}
            for e, p in ((0, p0), (1, p1)):
                z_pss[e] = ps2.tile([dh + 1, s_len], f32, name=f"zps_{p}", tag="ps2")
                zT_sbs2[e] = zT_pool.tile([128, s_len], bf16, name=f"zT_{p}", tag="zT")
                lf_sbs2[e] = lf_pool.tile([1, s_len], bf16, name=f"lf_{p}", tag="lf")
                recip_sbs2[e] = recip_pool.tile([128, NSQ], f32, name=f"recip_{p}", tag="recip")

            for i0 in range(0, NSQ, 2):
                blocks0 = _strip_blocks(i0, s_len)
                blocks1 = _strip_blocks(i0 + 1, s_len)
                nblk = max(len(blocks0), len(blocks1))
                sc_tiles = {}
                # row-packed score matmuls for BOTH pairs
                for e, p in ((0, p0), (1, p1)):
                    qkT_sb, swap_sb = qkTs[e], swaps[e]
                    for bi in range(nblk):
                        for di, i, blocks in ((0, i0, blocks0), (1, i0 + 1, blocks1)):
                            bj = bi - (nblk - len(blocks))
                            if bj < 0:
                                continue
                            a, b = blocks[bj]
                            sc_ps = scps.tile([128, 512], f32, name=f"sc_{p}_{i}_{a}", tag="scps")
                            if di == 0:
                                lhsT = swap_sb[0:dh, i * 128:(i + 1) * 128]
                                rhs = qkT_sb[0:dh, a:b]
                            else:
                                lhsT = qkT_sb[dh:2 * dh, i * 128:(i + 1) * 128]
                                rhs = swap_sb[dh:2 * dh, a:b]
                            nc.tensor.matmul(
                                sc_ps[:, 0:b - a], lhsT=lhsT, rhs=rhs,
                                start=True, stop=True,
                            )
                            sc_tiles[(e, i, a)] = sc_ps

                # PE gap fillers: deferred out-couples run here
                emit_one_pending()
                emit_one_pending()

                # exp (ScalarE), diag mask (GpSimd), z matmuls, extraction
                for e, p in ((0, p0), (1, p1)):
                    vaug_sb = vaugs[e]
                    z_ps = z_pss[e]
                    zT_sb, lf_sb, recip_sb = zT_sbs2[e], lf_sbs2[e], recip_sbs2[e]
                    for di, i, blocks in ((0, i0, blocks0), (1, i0 + 1, blocks1)):
                        for (a, b) in blocks:
                            sc_ps = sc_tiles[(e, i, a)]
                            pt_sb = pstrip_pool.tile([128, 512], bf16, name=f"pt_{p}_{i}_{a}", tag="pstrip")
                            nc.scalar.activation(pt_sb[:, 0:b - a], sc_ps[:, 0:b - a], Exp)
                            if a == 128 * i:  # leading block holds the diag triangle
                                dst = pt_sb[:, 0:128]
                                nc.gpsimd.affine_select(
                                    out=dst, in_=dst,
                                    compare_op=mybir.AluOpType.is_ge,
                                    fill=0.0, base=0,
                                    pattern=[[1, 128]], channel_multiplier=-1,
                                )
                            nc.tensor.matmul(
                                z_ps[:, a:b],
                                lhsT=vaug_sb[:, i * (dh + 1):(i + 1) * (dh + 1)],
                                rhs=pt_sb[:, 0:b - a],
                                start=(i == 0), stop=(i == (b - 1) // 128),
                                skip_group_check=True,
                            )

                    # eager extraction: z cols [128*i0, 128*i0+256) are final
                    c0, c1 = 128 * i0, 128 * (i0 + 2)
                    nc.vector.tensor_copy(zT_sb[0:dh, c0:c1], z_ps[0:dh, c0:c1])
                    if i0 in (2, 6):
                        # per-half: dup zT rows for row packing, extract l,
                        # compute recips, then queue 2 out-couples (lag 2)
                        h0, h1 = 512 * (i0 // 4), 512 * (i0 // 4) + 512
                        nc.gpsimd.dma_start(zT_sb[dh:2 * dh, h0:h1], zT_sb[0:dh, h0:h1])
                        nc.vector.tensor_copy(lf_sb[:, h0:h1], z_ps[dh:dh + 1, h0:h1])
                        l_ps = ops_pool.tile([128, 4], f32, name=f"lps_{p}_{i0}", tag="ops")
                        for dj in range(4):
                            j = i0 - 2 + dj
                            nc.tensor.matmul(
                                l_ps[:, dj:dj + 1],
                                lhsT=lf_sb[:, j * 128:(j + 1) * 128],
                                rhs=ones_sb[:, :],
                                start=True, stop=True,
                            )
                        nc.vector.reciprocal(recip_sb[:, i0 - 2:i0 + 2], l_ps[:])
                        pending_out.append(make_out(p, i0 - 2, zT_sb, recip_sb, wo_sb))
                        pending_out.append(make_out(p, i0, zT_sb, recip_sb, wo_sb))

        final_split[0] = True
        while pending_out:
            emit_one_pending()

    nc.finalize()
    _dedup_ldweights(nc, mybir)
    return nc


def _dedup_ldweights(nc, mybir):
    """Remove back-to-back duplicate Ldweights on the PE stream.

    bacc lowers every matmul to an Ldweights+Matmult pair and walrus runs
    with --enable-ldw-opt=false, so consecutive matmuls sharing a stationary
    operand reload it (~107 ns each).  Emission order makes same-weight
    matmuls adjacent; drop an Ldweights when it exactly repeats the previous
    one on the PE stream and carries no semaphore waits/updates."""
    pe = mybir.EngineType.PE
    removed = 0

    def footprint(inst):
        pos = getattr(inst, "tile_position", None) or (0, 0)
        size = getattr(inst, "tile_size", None) or (128, 128)
        return (pos[0], pos[0] + size[0], pos[1], pos[1] + size[1])

    def overlaps(a, b):
        return a[0] < b[1] and b[0] < a[1] and a[2] < b[3] and b[2] < a[3]

    for fn in nc.m.functions:
        for blk in fn.blocks:
            last = {}  # (pos, size) -> (sig, footprint)
            keep = []
            for inst in blk.instructions:
                if getattr(inst, "engine", None) == pe:
                    if isinstance(inst, mybir.InstLdweights):
                        key = (
                            repr(getattr(inst, "tile_position", None)),
                            repr(getattr(inst, "tile_size", None)),
                        )
                        sig = (
                            repr(inst.ins), repr(inst.perf_mode),
                            repr(inst.is_transpose),
                        )
                        si = inst.sync_info
                        syncfree = si is None or (not si.on_wait and not si.on_update)
                        prev = last.get(key)
                        fp = footprint(inst)
                        if prev is not None and prev[0] == sig and syncfree:
                            removed += 1
                            continue
                        # a new load invalidates any tracked load whose
                        # quadrant footprint it overwrites
                        for k in list(last):
                            if k != key and overlaps(last[k][1], fp):
                                del last[k]
                        last[key] = (sig, fp)
                    elif not isinstance(inst, mybir.InstMatmult):
                        last = {}
                keep.append(inst)
            if removed:
                del blk.instructions[:]
                for inst in keep:
                    blk.instructions.append(inst)
    return removed


E4M3 = ml_dtypes.float8_e4m3fn


def _split_fp8(a):
    """hi + lo fp8 decomposition: hi = e4m3(a), lo = e4m3(a - hi)."""
    hi = np.asarray(a, dtype=np.float32).astype(E4M3)
    lo = (np.asarray(a, dtype=np.float32) - hi.astype(np.float32)).astype(E4M3)
    return hi, lo


def prepare_shards(normalized_resid_pre, W_Q, b_Q, W_K, b_K, W_V, b_V, W_O, b_O):
    """Host-side layout: returns in_maps for the 8 cores."""
    x = np.asarray(normalized_resid_pre, dtype=np.float32)
    scale = 1.0 / np.sqrt(DH)
    KC = DM // 128

    pair_map = _core_pair_map()

    # x^T per (core, slot), partition-major: [128, KC*S]
    xt_f = x.transpose(0, 2, 3, 1)  # [B, H, DM, S]
    # W_Q pre-scaled by 1/sqrt(DH) so scores come out pre-scaled
    wqk_h = np.concatenate([np.asarray(W_Q) * scale, np.asarray(W_K)], axis=-1)
    wv_h = np.asarray(W_V)  # [H, DM, DH]
    wo_h = np.asarray(W_O)  # [H, DH, DM]

    ident = np.eye(128).astype(BF16)

    in_maps = []
    for c in range(N_CORES):
        xts, wqkvs, wos = [], [], []
        for s in range(PPC):
            b, h = pair_map[c][s]
            xts.append(
                xt_f[b, h].reshape(KC, 128, S).transpose(1, 0, 2).reshape(128, KC * S))
            if s % 2 == 0:
                wqkvs.append(np.concatenate(
                    [wqk_h[h].reshape(KC, 128, 2 * DH).transpose(1, 0, 2).reshape(128, KC * 2 * DH),
                     wv_h[h].reshape(KC, 128, DH).transpose(1, 0, 2).reshape(128, KC * DH)],
                    axis=1))
                wos.append(np.concatenate([wo_h[h], wo_h[h]], axis=0))  # [128, DM]
        in_maps.append({
            "xt": np.ascontiguousarray(np.stack(xts)).astype(BF16),
            "wqkv": np.ascontiguousarray(np.stack(wqkvs)).astype(BF16),
            "wo": np.ascontiguousarray(np.stack(wos)).astype(BF16),
            "ident": ident,
        })
    return in_maps


def _ensure_profile_hook():
    """The agent image lacks ``antenv.axon_hooks``; shim it and install the
    ctypes NTFF hook from trn_boot so trace=True works under axon."""
    import importlib
    import sys
    import types
    try:
        importlib.import_module("antenv.axon_hooks")
        return True
    except ImportError:
        pass
    try:
        import antenv
        mod = types.ModuleType("antenv.axon_hooks")
        _state = {"hook": None}
        mod.set_axon_ntff_profile_hook = lambda h: _state.__setitem__("hook", h)
        mod.get_axon_ntff_profile_hook = lambda: _state["hook"]
        sys.modules["antenv.axon_hooks"] = mod
        antenv.axon_hooks = mod
        from trn_agent_boot.trn_boot import _ntff_profile_via_ctypes
        hook = _ntff_profile_via_ctypes("/opt/axon/libaxon_pjrt.so")
        if hook is not None:
            mod.set_axon_ntff_profile_hook(hook)
        return hook is not None
    except Exception:
        return False


def kernel(**inputs):
    global LAST_EXEC_TIME_NS, LAST_RESULTS
    from concourse.bass_utils import run_bass_kernel_spmd

    in_maps = prepare_shards(**inputs)
    nc = build_nc()

    trace = bool(int(os.environ.get("KERNEL_PROFILE", "0")))
    tmpdir = None
    if trace:
        trace = _ensure_profile_hook()
        if trace:
            tmpdir = os.environ.get("KERNEL_PROFILE_DIR") or None
    res = run_bass_kernel_spmd(nc, in_maps, list(range(N_CORES)), trace=trace,
                               tmpdir=tmpdir)
    LAST_EXEC_TIME_NS = res.exec_time_ns
    LAST_RESULTS = res

    pair_map = _core_pair_map()
    out = np.empty((B, S, H, DM), dtype=np.float32)
    for c in range(N_CORES):
        dev = np.asarray(res.results[c]["out"], dtype=np.float32)
        # [PPC, S//256, 128, 2*DM] -> [PPC, S, DM]
        dev = (dev.reshape(PPC, S // 256, 128, 2, DM)
               .transpose(0, 1, 3, 2, 4).reshape(PPC, S, DM))
        for s in range(PPC):
            b, h = pair_map[c][s]
            out[b, :, h, :] = dev[s]

    b_O = np.asarray(inputs["b_O"], dtype=np.float32)
    b_V = np.asarray(inputs["b_V"], dtype=np.float32)
    b_Q = np.asarray(inputs["b_Q"], dtype=np.float32)
    b_K = np.asarray(inputs["b_K"], dtype=np.float32)
    if np.any(b_Q) or np.any(b_K):
        raise NotImplementedError("nonzero b_Q/b_K not supported by this kernel")
    extra = b_O[None, :] / H  # [1, DM] broadcast over heads
    if np.any(b_V):
        extra = extra + np.einsum(
            "hd,hdm->hm", b_V, np.asarray(inputs["W_O"], dtype=np.float32))
    if np.any(extra):
        out = out + extra[None, None]
    return np.ascontiguousarray(out, dtype=np.float32)
